# revision 24
# baseline (speedup 1.0000x reference)
"""
Trainium2 Bass kernel for nn_MoE_72808285602017 (segment_reduce).

Computes the PropertyLossTracker step: segment sums/counts of 16.7M token
losses into P=4096 property bins, EMA frequency update, inverse-frequency
weights, stratified + unweighted loss.

Strategy (data-parallel over 8 NeuronCores):
  - Shard the N=16,777,216 tokens across 8 cores (2,097,152 each).
  - Per core, segment-sum via the rank-2 one-hot matmul trick:
      id = hi*32 + lo (hi = id>>5 in [0,128), lo = id&31 in [0,32))
      A[t, h]  = (hi_t == h)          (bf16 one-hot, 128 wide)
      R[t, :]  = [ (lo_t==l)*loss_t | (lo_t==l) ]   (bf16, 2*32 wide)
      PSUM[128,64] += A^T @ R  accumulated over all 128-token chunks
      -> PSUM[h, 0:32] = sums[h*32+l], PSUM[h, 32:64] = counts[h*32+l]
  - AllReduce the [128,64] partials across the 8 cores.
  - Replicated tiny [P] EMA/weight math on-device; core 0's output is used.
"""

import sys

sys.path.insert(0, "/opt/trn_rl_repo")

import numpy as np

from concourse import bacc, bass_utils, mybir, tile
from concourse.mybir import AluOpType as OP

P = 4096
N = 16_777_216
NCORES = 8
TOK = N // NCORES          # tokens per core
HI = 128                   # hi bins  (id >> 5)
LO = 32                    # lo bins  (id & 31)

EMA_DECAY = 0.99
MIN_FREQ = 1e-5
WARMUP_BATCHES = 1000.0
SLOW_WARMUP = 3000.0
MAX_WEIGHT = 30.0
RAMP_BATCHES = 200.0

F32 = mybir.dt.float32
I32 = mybir.dt.int32
BF16 = mybir.dt.bfloat16


def _body(tc, nc, ids_d, loss_d, freq_d, bc_d, ofreq_d, oscal_d, tok, f_tile,
          c_batch, mode="stt", skip=""):
    n_total = tok * NCORES
    tpp = tok // 128            # tokens per partition
    outer = tpp // f_tile       # outer DMA tiles
    nb = f_tile // c_batch      # build batches per outer tile

    ids2 = ids_d.rearrange("(p f) -> p f", p=128)
    loss2 = loss_d.rearrange("(p f) -> p f", p=128)

    from contextlib import ExitStack
    ctx = ExitStack()
    const = ctx.enter_context(tc.tile_pool(name="const", bufs=1))
    io_pool = ctx.enter_context(tc.tile_pool(name="io", bufs=2))
    pre_pool = ctx.enter_context(tc.tile_pool(name="pre", bufs=2))
    nbufs = 3 if mode == "stt2" else 4
    a_pool = ctx.enter_context(tc.tile_pool(name="abuild", bufs=nbufs))
    r_pool = ctx.enter_context(tc.tile_pool(name="rbuild", bufs=nbufs))
    ps_pool = ctx.enter_context(tc.tile_pool(name="psum", bufs=2, space="PSUM"))
    fin = ctx.enter_context(tc.tile_pool(name="fin", bufs=1))
    dram = ctx.enter_context(tc.tile_pool(name="dram", bufs=1, space="DRAM"))

    # --- constants ---
    iota_i = const.tile([128, HI], I32, tag="iota_i")
    nc.gpsimd.iota(iota_i[:], pattern=[[1, HI]], base=0, channel_multiplier=0)
    iota_bf = const.tile([128, HI], BF16, tag="iota_bf")
    nc.vector.tensor_copy(iota_bf[:], iota_i[:])

    if mode == "stt2":
        # replicated iotas with chunk as the inner dim: value(h, c) = h.
        # bf16 is exact for integers < 256, so generate directly.
        iota_hi_rep = const.tile([128, HI * c_batch], BF16, tag="ihr")
        nc.gpsimd.iota(iota_hi_rep[:], pattern=[[1, HI], [0, c_batch]],
                       base=0, channel_multiplier=0,
                       allow_small_or_imprecise_dtypes=True)
        iota_lo_rep = const.tile([128, LO * c_batch], BF16, tag="ilr")
        nc.gpsimd.iota(iota_lo_rep[:], pattern=[[1, LO], [0, c_batch]],
                       base=0, channel_multiplier=0,
                       allow_small_or_imprecise_dtypes=True)

    acc = const.tile([2 * LO, HI], F32, tag="acc")
    nc.vector.memset(acc[:], 0.0)

    # 64x64 f32 identity for the post-collective PE transpose
    ident_i = const.tile([64, 64], I32, tag="ident_i")
    nc.gpsimd.iota(ident_i[:], pattern=[[1, 64]], base=0, channel_multiplier=-1)
    ident_f = const.tile([64, 64], F32, tag="ident_f")
    nc.vector.tensor_copy(ident_f[:], ident_i[:])
    nc.vector.tensor_scalar(ident_f[:], ident_f[:], 0.0, None, OP.is_equal)

    # --- main loop: build one-hots, matmul-accumulate ---
    for o in range(outer):
        ids_t = io_pool.tile([128, f_tile], I32, tag="ids")
        nc.sync.dma_start(ids_t[:], ids2[:, o * f_tile:(o + 1) * f_tile])
        loss_t = io_pool.tile([128, f_tile], F32, tag="loss")
        nc.sync.dma_start(loss_t[:], loss2[:, o * f_tile:(o + 1) * f_tile])

        pre_dt = F32 if mode == "ts" else BF16
        hi_i = pre_pool.tile([128, f_tile], I32, tag="scr_i")
        nc.vector.tensor_scalar(hi_i[:], ids_t[:], 5, None, OP.logical_shift_right)
        hi_b = pre_pool.tile([128, f_tile], pre_dt, tag="hi_b")
        nc.vector.tensor_copy(hi_b[:], hi_i[:])
        lo_i = pre_pool.tile([128, f_tile], I32, tag="scr_i")
        nc.vector.tensor_scalar(lo_i[:], ids_t[:], 31, None, OP.bitwise_and)
        lo_b = pre_pool.tile([128, f_tile], pre_dt, tag="lo_b")
        nc.vector.tensor_copy(lo_b[:], lo_i[:])
        if mode == "stt":
            loss_b = pre_pool.tile([128, f_tile], BF16, tag="loss_b")
            nc.vector.tensor_copy(loss_b[:], loss_t[:])
        else:
            loss_b = loss_t

        ps = ps_pool.tile([2 * LO, HI], F32, tag="ps")

        for b in range(nb):
            j0 = b * c_batch
            a_t = a_pool.tile([128, c_batch * HI], BF16, tag="a")
            r_t = r_pool.tile([128, c_batch * 2 * LO], BF16, tag="r")

            if skip == "dve":
                nc.vector.memset(a_t[:, 0:2], 0.0)
                nc.vector.memset(r_t[:, 0:2], 0.0)
            elif mode == "stt2":
                # layout [p, bin, chunk]: chunk is innermost (step 1) so both
                # tensor_tensor operands qualify for the 2x_1p DVE mode.
                a3 = a_t[:].rearrange("p (h c) -> p h c", c=c_batch)
                r3 = r_t[:].rearrange("p (n c) -> p n c", c=c_batch)
                hi_v = hi_b[:, j0:j0 + c_batch].unsqueeze(1).broadcast_to(
                    [128, HI, c_batch])
                nc.vector.tensor_tensor(
                    a3, hi_v,
                    iota_hi_rep[:].rearrange("p (h c) -> p h c", c=c_batch),
                    OP.is_equal)
                lo_v = lo_b[:, j0:j0 + c_batch].unsqueeze(1).broadcast_to(
                    [128, LO, c_batch])
                nc.vector.tensor_tensor(
                    r3[:, LO:2 * LO, :], lo_v,
                    iota_lo_rep[:].rearrange("p (l c) -> p l c", c=c_batch),
                    OP.is_equal)
                loss_v = loss_b[:, j0:j0 + c_batch].unsqueeze(1).broadcast_to(
                    [128, LO, c_batch])
                nc.vector.tensor_tensor(
                    r3[:, 0:LO, :], r3[:, LO:2 * LO, :], loss_v, OP.mult)
            elif mode == "stt":
                a3 = a_t[:].rearrange("p (c m) -> p c m", m=HI)
                r3 = r_t[:].rearrange("p (c n) -> p c n", n=2 * LO)

                hi_v = hi_b[:, j0:j0 + c_batch].unsqueeze(2).broadcast_to(
                    [128, c_batch, HI])
                iota_hi_v = iota_bf[:].unsqueeze(1).broadcast_to(
                    [128, c_batch, HI])
                nc.vector.scalar_tensor_tensor(
                    a3, hi_v, 0.0, iota_hi_v, OP.add, OP.is_equal)

                lo_v = lo_b[:, j0:j0 + c_batch].unsqueeze(2).broadcast_to(
                    [128, c_batch, LO])
                iota_lo_v = iota_bf[:, :LO].unsqueeze(1).broadcast_to(
                    [128, c_batch, LO])
                nc.vector.scalar_tensor_tensor(
                    r3[:, :, LO:2 * LO], lo_v, 0.0, iota_lo_v,
                    OP.add, OP.is_equal)

                loss_v = loss_b[:, j0:j0 + c_batch].unsqueeze(2).broadcast_to(
                    [128, c_batch, LO])
                nc.vector.scalar_tensor_tensor(
                    r3[:, :, 0:LO], r3[:, :, LO:2 * LO], 0.0, loss_v,
                    OP.add, OP.mult)
            else:  # mode == "ts": per-chunk tensor_scalar (4x-mode candidates)
                for c in range(c_batch):
                    j = j0 + c
                    nc.vector.tensor_scalar(
                        a_t[:, c * HI:(c + 1) * HI], iota_bf[:],
                        hi_b[:, j:j + 1], None, OP.is_equal)
                    nc.vector.tensor_scalar(
                        r_t[:, c * 64:c * 64 + 32], iota_bf[:, :LO],
                        lo_b[:, j:j + 1], loss_b[:, j:j + 1],
                        OP.is_equal, OP.mult)
                    nc.vector.tensor_scalar(
                        r_t[:, c * 64 + 32:c * 64 + 64], iota_bf[:, :LO],
                        lo_b[:, j:j + 1], None, OP.is_equal)

            if skip != "pe":
                if mode == "stt2":
                    a_sl = a_t[:].rearrange("p (h c) -> p h c", c=c_batch)
                    r_sl = r_t[:].rearrange("p (n c) -> p n c", c=c_batch)
                for c in range(c_batch):
                    first = b == 0 and c == 0
                    last = b == nb - 1 and c == c_batch - 1
                    # out[m, h] = sum_t R[t, m] * A[t, h]: lhsT=R (64-col
                    # weight load) hides under rhs=A's 128-col matmul stream.
                    if mode == "stt2":
                        lhs = r_sl[:, :, c:c + 1].squeeze(2)
                        rhs = a_sl[:, :, c:c + 1].squeeze(2)
                    else:
                        lhs = r_t[:, c * 2 * LO:(c + 1) * 2 * LO]
                        rhs = a_t[:, c * HI:(c + 1) * HI]
                    nc.tensor.matmul(ps[:], lhs, rhs, start=first, stop=last)

        if skip != "pe":
            nc.vector.tensor_tensor(acc[:], acc[:], ps[:], OP.add)

    # --- all-reduce partials across the 8 cores ---
    bounce_in = dram.tile([2 * LO, HI], F32, tag="cc_in")
    bounce_out = dram.tile([2 * LO, HI], F32, tag="cc_out")
    nc.sync.dma_start(bounce_in[:], acc[:])
    nc.gpsimd.collective_compute(
        "AllReduce",
        OP.add,
        ins=[bounce_in.opt()],
        outs=[bounce_out.opt()],
        replica_groups=[list(range(NCORES))],
    )
    red0 = fin.tile([2 * LO, HI], F32, tag="red0")
    nc.sync.dma_start(red0[:], bounce_out[:])

    # transpose [64, 128] -> [128, 64] so property p = partition*32 + col
    ps_t = ps_pool.tile([HI, 2 * LO], F32, tag="ps_t")
    nc.tensor.transpose(ps_t[:], red0[:], ident_f[:])
    red = fin.tile([HI, 2 * LO], F32, tag="red")
    nc.vector.tensor_copy(red[:], ps_t[:])

    sums = red[:, 0:LO]
    counts = red[:, LO:2 * LO]

    # --- replicated final math on [128, 32] tiles (property p = part*32+col) ---
    freq_sb = fin.tile([128, LO], F32, tag="freq")
    nc.sync.dma_start(freq_sb[:], freq_d.rearrange("(p l) -> p l", l=LO))
    bc_t = fin.tile([128, 1], F32, tag="bc")
    nc.sync.dma_start(bc_t[:], bc_d.rearrange("(p l) -> p l", l=1))

    ones32 = fin.tile([128, LO], F32, tag="ones32")
    nc.vector.memset(ones32[:], 1.0)

    present = fin.tile([128, LO], F32, tag="present")
    nc.vector.tensor_scalar(present[:], counts, 1.0, None, OP.min)
    denom = fin.tile([128, LO], F32, tag="denom")
    nc.vector.tensor_scalar(denom[:], counts, 1.0, None, OP.max)
    rden = fin.tile([128, LO], F32, tag="rden")
    nc.vector.reciprocal(rden[:], denom[:])
    mean_loss = fin.tile([128, LO], F32, tag="mean_loss")
    nc.vector.tensor_tensor(mean_loss[:], sums, rden[:], OP.mult)

    # new_freq = 0.99*freq + 0.01*counts/(n + 1e-6)
    kf = (1.0 - EMA_DECAY) / (float(n_total) + 1e-6)
    cnt_sc = fin.tile([128, LO], F32, tag="cnt_sc")
    nc.vector.tensor_scalar(cnt_sc[:], counts, kf, None, OP.mult)
    new_freq = fin.tile([128, LO], F32, tag="new_freq")
    nc.vector.scalar_tensor_tensor(
        new_freq[:], freq_sb[:], EMA_DECAY, cnt_sc[:], OP.mult, OP.add)

    # raw = (max(new_freq, MIN_FREQ) + 1e-6) ** -0.5
    fq = fin.tile([128, LO], F32, tag="fq")
    nc.vector.tensor_scalar(fq[:], new_freq[:], MIN_FREQ, 1e-6, OP.max, OP.add)
    sq = fin.tile([128, LO], F32, tag="sq")
    nc.scalar.activation(sq[:], fq[:], mybir.ActivationFunctionType.Sqrt)
    raw = fin.tile([128, LO], F32, tag="raw")
    nc.vector.reciprocal(raw[:], sq[:])

    # ramp = min(1, (bc-1000)/200);  raw = 1 + ramp*(raw-1);  raw = min(30, raw)
    ramp = fin.tile([128, 1], F32, tag="ramp")
    nc.vector.tensor_scalar(
        ramp[:], bc_t[:], 1.0 / RAMP_BATCHES, -WARMUP_BATCHES / RAMP_BATCHES,
        OP.mult, OP.add)
    nc.vector.tensor_scalar(ramp[:], ramp[:], 1.0, None, OP.min)
    rm1 = fin.tile([128, LO], F32, tag="rm1")
    nc.vector.tensor_scalar(rm1[:], raw[:], -1.0, None, OP.add)
    raw2 = fin.tile([128, LO], F32, tag="raw2")
    nc.vector.scalar_tensor_tensor(
        raw2[:], rm1[:], ramp[:], ones32[:], OP.mult, OP.add)
    nc.vector.tensor_scalar(raw2[:], raw2[:], MAX_WEIGHT, None, OP.min)

    # slow-warmup blend: if bc <= 3000: raw = raw*frac + (1-frac), frac = bc/3000
    frac = fin.tile([128, 1], F32, tag="frac")
    nc.vector.tensor_scalar(frac[:], bc_t[:], 1.0 / SLOW_WARMUP, None, OP.mult)
    blend = fin.tile([128, LO], F32, tag="blend")
    # blend = raw2*frac - frac + 1 = (raw2-1)*frac + 1
    r2m1 = fin.tile([128, LO], F32, tag="r2m1")
    nc.vector.tensor_scalar(r2m1[:], raw2[:], -1.0, None, OP.add)
    nc.vector.scalar_tensor_tensor(
        blend[:], r2m1[:], frac[:], ones32[:], OP.mult, OP.add)
    sblend = fin.tile([128, 1], F32, tag="sblend")
    nc.vector.tensor_scalar(sblend[:], bc_t[:], SLOW_WARMUP, None, OP.is_le)
    diff = fin.tile([128, LO], F32, tag="diff")
    nc.vector.tensor_tensor(diff[:], blend[:], raw2[:], OP.subtract)
    raw4 = fin.tile([128, LO], F32, tag="raw4")
    nc.vector.scalar_tensor_tensor(
        raw4[:], diff[:], sblend[:], raw2[:], OP.mult, OP.add)

    # warmup: if bc <= 1000: raw = 1
    wmask = fin.tile([128, 1], F32, tag="wmask")
    nc.vector.tensor_scalar(wmask[:], bc_t[:], WARMUP_BATCHES, None, OP.is_le)
    omr = fin.tile([128, LO], F32, tag="omr")
    nc.vector.scalar_tensor_tensor(
        omr[:], raw4[:], -1.0, ones32[:], OP.mult, OP.add)  # 1 - raw4
    raw5 = fin.tile([128, LO], F32, tag="raw5")
    nc.vector.scalar_tensor_tensor(
        raw5[:], omr[:], wmask[:], raw4[:], OP.mult, OP.add)

    w = fin.tile([128, LO], F32, tag="w")
    nc.vector.tensor_tensor(w[:], raw5[:], present[:], OP.mult)
    mw = fin.tile([128, LO], F32, tag="mw")
    nc.vector.tensor_tensor(mw[:], mean_loss[:], w[:], OP.mult)

    # cross-partition sums of [w, mean_loss*w, sums] via ones-matmul
    red3 = fin.tile([128, 3], F32, tag="red3")
    nc.vector.tensor_reduce(red3[:, 0:1], w[:], mybir.AxisListType.X, OP.add)
    nc.vector.tensor_reduce(red3[:, 1:2], mw[:], mybir.AxisListType.X, OP.add)
    nc.vector.tensor_reduce(red3[:, 2:3], sums, mybir.AxisListType.X, OP.add)
    ones1 = fin.tile([128, 1], F32, tag="ones1")
    nc.vector.memset(ones1[:], 1.0)
    ps_s = ps_pool.tile([3, 1], F32, tag="ps_s")
    nc.tensor.matmul(ps_s[:], red3[:], ones1[:], start=True, stop=True)

    # bring the 3 partition-scalars onto one partition via DRAM roundtrip
    ps_sb = fin.tile([3, 1], F32, tag="ps_sb")
    nc.vector.tensor_copy(ps_sb[:], ps_s[:])
    scal_dram = dram.tile([1, 3], F32, tag="scal")
    nc.sync.dma_start(scal_dram[:].rearrange("o (p l) -> (o p) l", l=1), ps_sb[:])
    scal = fin.tile([1, 3], F32, tag="scal_sb")
    nc.sync.dma_start(scal[:], scal_dram[:])

    wsum_r = fin.tile([1, 1], F32, tag="wsum_r")
    nc.vector.tensor_scalar(wsum_r[:], scal[:, 0:1], 1e-6, None, OP.add)
    nc.vector.reciprocal(wsum_r[:], wsum_r[:])
    out_sc = fin.tile([1, 2], F32, tag="out_sc")
    nc.vector.tensor_tensor(out_sc[:, 0:1], scal[:, 1:2], wsum_r[:], OP.mult)
    nc.vector.tensor_scalar(out_sc[:, 1:2], scal[:, 2:3], 1.0 / float(n_total),
                            None, OP.mult)

    # outputs
    nc.sync.dma_start(ofreq_d.rearrange("(p l) -> p l", l=LO), new_freq[:])
    nc.sync.dma_start(oscal_d.rearrange("(o l) -> o l", o=1), out_sc[:])

    ctx.close()


def build(tok=TOK, f_tile=2048, c_batch=64, enable_asserts=False,
          mode="stt", skip=""):
    nc = bacc.Bacc(
        "TRN2",
        target_bir_lowering=False,
        debug=False,
        enable_asserts=enable_asserts,
        num_devices=NCORES,
    )
    ids_d = nc.dram_tensor("ids", [tok], I32, kind="ExternalInput").ap()
    loss_d = nc.dram_tensor("losses", [tok], F32, kind="ExternalInput").ap()
    freq_d = nc.dram_tensor("freq", [P], F32, kind="ExternalInput").ap()
    bc_d = nc.dram_tensor("bc", [128], F32, kind="ExternalInput").ap()
    ofreq_d = nc.dram_tensor("out_freq", [P], F32, kind="ExternalOutput").ap()
    oscal_d = nc.dram_tensor("out_scalars", [2], F32, kind="ExternalOutput").ap()
    with tile.TileContext(nc) as tc:
        _body(tc, nc, ids_d, loss_d, freq_d, bc_d, ofreq_d, oscal_d,
              tok, f_tile, c_batch, mode=mode, skip=skip)
    nc.compile()
    return nc


_NC_CACHE = {}


def _get_nc():
    if "nc" not in _NC_CACHE:
        _NC_CACHE["nc"] = build()
    return _NC_CACHE["nc"]


def run_on_device(nc, in_maps, **kwargs):
    return bass_utils.run_bass_kernel_spmd(
        nc, in_maps, core_ids=list(range(NCORES)), **kwargs)


def make_in_maps(property_ids, token_losses, prop_freq, batch_counter, tok=TOK):
    ids = np.asarray(property_ids).astype(np.int32, copy=False)
    losses = np.asarray(token_losses).astype(np.float32, copy=False)
    freq = np.asarray(prop_freq).astype(np.float32, copy=False)
    bcv = float(np.asarray(batch_counter))
    n = ids.shape[0]
    ncores = n // tok
    ids_sh = ids.reshape(ncores, tok)
    loss_sh = losses.reshape(ncores, tok)
    bc_arr = np.full((128,), bcv, np.float32)
    return [
        dict(ids=np.ascontiguousarray(ids_sh[i]),
             losses=np.ascontiguousarray(loss_sh[i]),
             freq=freq, bc=bc_arr)
        for i in range(ncores)
    ]


def kernel(property_ids, token_losses, prop_freq, batch_counter):
    in_maps = make_in_maps(property_ids, token_losses, prop_freq, batch_counter)
    res = run_on_device(_get_nc(), in_maps)
    out = res.results[0]
    s = np.asarray(out["out_scalars"], dtype=np.float32)
    f = np.asarray(out["out_freq"], dtype=np.float32)
    return np.float32(s[0]), np.float32(s[1]), f
